# revision 16
# baseline (speedup 1.0000x reference)
"""GQA attention block (RMSNorm-QK + RoPE + causal attention + proj) on 8 TRN2 cores.

Sharding: DP=2 over batch x TP=4 over heads (4 q heads + 1 kv head per core).

v2: chunk-pipelined emission (QKV(ch) -> attention(quad ch) -> AllGather ->
lagged proj) so ACT/DVE streams hide under PE work; v-stationary AV matmuls
producing y^T directly (no output transposes); rowsum via per-tile ones-matmul
(PE) or P_sum accumulation (DVE) per RS_DVE knob; 5-op RoPE with the -sin sign
baked into the host table.
"""

import math
import os
import sys

import numpy as np

for _p in ("/opt/trn_rl_repo", "/root/.axon_site/_ro/trn_rl_repo"):
    if os.path.isdir(_p) and _p not in sys.path:
        sys.path.insert(0, _p)

import ml_dtypes

import concourse.bacc as bacc
import concourse.mybir as mybir
import concourse.tile as tile
from concourse import masks
from concourse.bass_utils import run_bass_kernel_spmd

BF16 = mybir.dt.bfloat16
F32 = mybir.dt.float32
ALU = mybir.AluOpType
AF = mybir.ActivationFunctionType

B, T, C = 2, 2048, 2048
NH, NKV, HS = 16, 4, 128
TP = 4                # tensor-parallel ranks per batch element
QH = NH // TP         # q heads per core
QW = QH * HS          # 512
PT = 128
NT = T // PT          # 16
NCT = C // PT         # 16
NCH4 = 4              # t-chunks of 512
H2 = HS // 2
EPS = 1e-6
THETA = 10000.0
NCORES = 8
BF = ml_dtypes.bfloat16

RS_DVE = int(os.environ.get("BASS_RS_DVE", "2"))  # heads using DVE P_sum rowsum

_CACHE = {}


def _build(loop_r=None, ablate=(), rs_dve=None):
    if rs_dve is None:
        rs_dve = RS_DVE
    nc = bacc.Bacc(None, target_bir_lowering=False, num_devices=NCORES)

    xT = nc.declare_dram_parameter("xT", [C, T], BF16, isOutput=False)
    wq = nc.declare_dram_parameter("wq", [C, QW], BF16, isOutput=False)
    wkv = nc.declare_dram_parameter("wkv", [C, 2 * HS], BF16, isOutput=False)
    wp = nc.declare_dram_parameter("wp", [C, QW], BF16, isOutput=False)
    v1sT = nc.declare_dram_parameter("v1sT", [HS, T], F32, isOutput=False)
    cosqT = nc.declare_dram_parameter("cosqT", [HS, T], BF16, isOutput=False)
    sinqT = nc.declare_dram_parameter("sinqT", [HS, T], BF16, isOutput=False)
    coskT = nc.declare_dram_parameter("coskT", [HS, T], BF16, isOutput=False)
    sinkT = nc.declare_dram_parameter("sinkT", [HS, T], BF16, isOutput=False)
    mneg = nc.declare_dram_parameter("mneg", [PT, PT], BF16, isOutput=False)
    out = nc.declare_dram_parameter("out", [T, QW], F32, isOutput=True)

    groups = [[0, 1, 2, 3], [4, 5, 6, 7]]

    with tile.TileContext(nc) as tc:
        with (
            tc.tile_pool(name="const", bufs=1) as const,
            tc.tile_pool(name="persist", bufs=1) as persist,
            tc.tile_pool(name="psum", bufs=1, space="PSUM") as psum,
            tc.tile_pool(name="wk", bufs=3) as wk,
            tc.tile_pool(name="dram", bufs=1, space="DRAM") as dram,
        ):
            ident = const.tile([PT, PT], BF16)
            masks.make_identity(nc, ident[:])
            maskt = const.tile([PT, PT], BF16)
            nc.sync.dma_start(maskt[:], mneg[:])
            eps_t = const.tile([PT, 1], F32)
            nc.gpsimd.memset(eps_t[:], EPS)
            ones_t = const.tile([PT, 1], BF16)
            nc.gpsimd.memset(ones_t[:], 1.0)
            ones_f = const.tile([PT, 1], F32)
            nc.gpsimd.memset(ones_f[:], 1.0)
            ones1 = const.tile([1, PT], BF16)
            nc.gpsimd.memset(ones1[:], 1.0)

            # weights + rope tables resident in SBUF
            wkv_s = persist.tile([PT, NCT, 2 * HS], BF16)
            wq_s = persist.tile([PT, NCT, QW], BF16)
            wp_s = persist.tile([PT, NCT, QW], BF16)
            for ci in range(NCT):
                nc.sync.dma_start(wkv_s[:, ci, :], wkv[ci * PT:(ci + 1) * PT, :])
            cq_s = persist.tile([PT, T], BF16)
            sq_s = persist.tile([PT, T], BF16)
            ck_s = persist.tile([PT, T], BF16)
            sk_s = persist.tile([PT, T], BF16)
            nc.sync.dma_start(ck_s[:], coskT[:])
            nc.sync.dma_start(sk_s[:], sinkT[:])
            nc.sync.dma_start(cq_s[:], cosqT[:])
            nc.sync.dma_start(sq_s[:], sinqT[:])
            for ci in range(NCT):
                nc.sync.dma_start(wq_s[:, ci, :], wq[ci * PT:(ci + 1) * PT, :])
            for ci in range(NCT):
                nc.sync.dma_start(wp_s[:, ci, :], wp[ci * PT:(ci + 1) * PT, :])

            qT_s = persist.tile([PT, QH, T], BF16)
            kT_s = persist.tile([PT, T], BF16)
            v_s = persist.tile([PT, NT, HS], BF16)
            yT_s = persist.tile([PT, QH, T], BF16)

            TC = 256
            NCH = T // TC  # 8
            ag_ins = [dram.tile([QW, TC], BF16, name=f"ag_in{c}") for c in range(NCH)]
            ag_outs = [dram.tile([C, TC], BF16, name=f"ag_out{c}") for c in range(NCH)]

            def _qkv_mm(ch, u, xt_c):
                """u: 0=k, 1=v, 2..5 = q heads 0..3. Emit the GEMM only."""
                if u == 0:
                    wt, w0 = wkv_s, 0
                elif u == 1:
                    wt, w0 = wkv_s, HS
                else:
                    wt, w0 = wq_s, (u - 2) * HS
                zt = psum.tile([PT, 512], F32, tag="big", bufs=2, name=f"zt{ch}_{u}")
                for ci in range(NCT):
                    nc.tensor.matmul(
                        zt[:], wt[:, ci, w0:w0 + HS], xt_c[:, ci, :],
                        start=(ci == 0), stop=(ci == NCT - 1),
                    )
                return zt

            def _qkv_consume(ch, u, zt):
                c0 = ch * 512
                if u == 1:
                    # v: residual mix then transpose to keys-on-partitions
                    v1tt = wk.tile([PT, 512], F32, tag="v1tt", bufs=2)
                    nc.sync.dma_start(v1tt[:], v1sT[:, c0:c0 + 512])
                    vmx = wk.tile([PT, 512], BF16, tag="vmx", bufs=2)
                    nc.vector.tensor_tensor(vmx[:], zt[:], v1tt[:], ALU.add)
                    for b in range(4):
                        j = ch * 4 + b
                        tv = psum.tile([PT, PT], BF16, tag="big", bufs=2,
                                       name=f"tv{ch}_{b}")
                        nc.tensor.transpose(tv[:], vmx[:, b * PT:(b + 1) * PT],
                                            ident[:])
                        nc.vector.tensor_copy(v_s[:, j, :], tv[:])
                    return
                # q/k: rmsnorm stats -> scale -> rope
                sqv = wk.tile([PT, 512], BF16, tag="sqv", bufs=2)
                nc.scalar.square(sqv[:], zt[:])
                ms = psum.tile([1, 512], F32, tag="ms", bufs=1, name=f"ms{ch}_{u}")
                nc.tensor.matmul(ms[:], ones_t[:, 0:1], sqv[:], start=True, stop=True)
                rsT = wk.tile([1, 512], F32, tag="rsT", bufs=2)
                nc.scalar.activation(rsT[:], ms[:], AF.Abs_reciprocal_sqrt,
                                     bias=eps_t[0:1, :], scale=1.0 / HS)
                rsS = wk.tile([PT, 512], F32, tag="rsS", bufs=2)
                nc.gpsimd.partition_broadcast(rsS[:], rsT[:])
                zs = wk.tile([PT, 512], BF16, tag="zs", bufs=2)
                nc.vector.tensor_tensor(zs[:], zt[:], rsS[:], ALU.mult)
                ct, st = (ck_s, sk_s) if u == 0 else (cq_s, sq_s)
                kc = wk.tile([PT, 512], BF16, tag="kc", bufs=2)
                nc.vector.tensor_tensor(kc[:], zs[:], ct[:, c0:c0 + 512], ALU.mult)
                zr = wk.tile([PT, 512], BF16, tag="zr", bufs=2)
                nc.vector.tensor_copy(zr[0:H2, :], zs[H2:HS, :])
                nc.vector.tensor_copy(zr[H2:HS, :], zs[0:H2, :])
                tmp = wk.tile([PT, 512], BF16, tag="tmp", bufs=2)
                nc.vector.tensor_tensor(tmp[:], zr[:], st[:, c0:c0 + 512], ALU.mult)
                dst = kT_s[:, c0:c0 + 512] if u == 0 else qT_s[:, u - 2, c0:c0 + 512]
                nc.vector.tensor_tensor(dst, kc[:], tmp[:], ALU.add)

            def _attn_quad(ch, pending=None):
                q0 = ch * 512
                J = 4 * ch + 4
                seq = [(h, j) for h in range(QH) for j in range(J)]
                sps = {}
                st8 = {}

                def emit_score(h, j):
                    d = j - 4 * ch
                    w = 512 if d < 0 else (4 - d) * PT
                    qlo = q0 if d < 0 else j * PT
                    sp = psum.tile([PT, 512], F32, tag="big", bufs=2,
                                   name=f"sp{ch}_{h}_{j}")
                    if d >= 0:
                        # causal mask: -1e30 on the strictly-upper block, then
                        # the score matmul accumulates on top
                        nc.tensor.matmul(sp[:, 0:PT], ident[:], maskt[:],
                                         start=True, stop=False,
                                         skip_group_check=True)
                    nc.tensor.matmul(sp[:, 0:w], kT_s[:, j * PT:(j + 1) * PT],
                                     qT_s[:, h, qlo:q0 + 512],
                                     start=(d < 0), stop=True,
                                     skip_group_check=True)
                    sps[(h, j)] = sp

                emit_score(*seq[0])
                if pending is not None:
                    pending()
                for idx, (h, j) in enumerate(seq):
                    dve_rs = h >= QH - rs_dve
                    d = j - 4 * ch
                    d0 = max(d, 0)
                    w = 512 if d < 0 else (4 - d) * PT
                    if idx + 1 < len(seq):
                        emit_score(*seq[idx + 1])
                    sp = sps.pop((h, j))
                    if j == 0:
                        yp = psum.tile([PT, 512], F32, tag="yp", bufs=2,
                                       name=f"yp{ch}_{h}")
                        if dve_rs:
                            acc = wk.tile([PT, 512], F32, tag="ps", bufs=2,
                                          name=f"ps{ch}_{h}")
                        else:
                            acc = psum.tile([1, 512], F32, tag="rs", bufs=1,
                                            name=f"rsum{ch}_{h}")
                        st8[h] = (yp, acc)
                    yp, acc = st8[h]
                    pts = wk.tile([PT, 512], BF16, tag="pts", bufs=3)
                    nc.scalar.activation(pts[:, 0:w], sp[:, 0:w], AF.Exp)
                    for r in range(d0, 4):
                        nc.tensor.matmul(
                            yp[:, r * PT:(r + 1) * PT], v_s[:, j, :],
                            pts[:, (r - d0) * PT:(r - d0 + 1) * PT],
                            start=(j == 0 and r == d0),
                            stop=(j == 4 * ch + r),
                            skip_group_check=True,
                        )
                    if dve_rs:
                        if j == 0:
                            nc.vector.tensor_copy(acc[:], pts[:])
                        else:
                            nc.vector.tensor_tensor(
                                acc[:, d0 * PT:], acc[:, d0 * PT:],
                                pts[:, 0:w], ALU.add)
                    else:
                        for r in range(d0, 4):
                            nc.tensor.matmul(
                                acc[:, r * PT:(r + 1) * PT], ones_t[:, 0:1],
                                pts[:, (r - d0) * PT:(r - d0 + 1) * PT],
                                start=(j == 0 and r == d0),
                                stop=(j == 4 * ch + r),
                                skip_group_check=True,
                            )
                    if j == J - 1:
                        # finalize head h
                        yp, acc = st8.pop(h)
                        if dve_rs:
                            rsum = psum.tile([1, 512], F32, tag="rs", bufs=1,
                                             name=f"rsum{ch}_{h}")
                            nc.tensor.matmul(rsum[:], ones_f[:, 0:1], acc[:],
                                             start=True, stop=True)
                        else:
                            rsum = acc
                        rinvT = wk.tile([1, 512], F32, tag="rinvT", bufs=2)
                        nc.vector.reciprocal(rinvT[:], rsum[:])
                        rinvS = wk.tile([PT, 512], F32, tag="rsS", bufs=2)
                        nc.gpsimd.partition_broadcast(rinvS[:], rinvT[:])
                        nc.vector.tensor_tensor(yT_s[:, h, q0:q0 + 512], yp[:],
                                                rinvS[:], ALU.mult)

            def _ag_issue(c):
                c0 = c * TC
                nc.sync.dma_start(
                    ag_ins[c][:].rearrange("(h p) t -> p h t", p=PT),
                    yT_s[:, :, c0:c0 + TC],
                )
                nc.gpsimd.collective_compute(
                    "AllGather", ALU.bypass, replica_groups=groups,
                    ins=[ag_ins[c][:]], outs=[ag_outs[c][:]],
                )

            def _proj_chunk(c, timing_only):
                ytf = wk.tile([PT, NCT, TC], BF16, tag="ytf", bufs=2)
                if timing_only:
                    nc.sync.dma_start(
                        ytf[:],
                        xT[:, c * TC:(c + 1) * TC].rearrange(
                            "(c2 p) t -> p c2 t", p=PT),
                    )
                else:
                    nc.sync.dma_start(
                        ytf[:],
                        ag_outs[c][:].rearrange("(c2 p) t -> p c2 t", p=PT),
                    )
                for tt in range(TC // PT):
                    ti = c * (TC // PT) + tt
                    pp = psum.tile([PT, QW], F32, tag="pp", bufs=2)
                    for ci in range(NCT):
                        nc.tensor.matmul(
                            pp[:], ytf[:, ci, tt * PT:(tt + 1) * PT],
                            wp_s[:, ci, :],
                            start=(ci == 0), stop=(ci == NCT - 1),
                        )
                    ot = wk.tile([PT, QW], F32, tag="ot", bufs=2)
                    nc.vector.tensor_copy(ot[:], pp[:])
                    nc.sync.dma_start(out[ti * PT:(ti + 1) * PT, :], ot[:])

            def _body(timing_only):
                done = []
                for ch in range(NCH4):
                    pending = None
                    if "A" not in ablate:
                        xt_c = wk.tile([PT, NCT, 512], BF16, tag="xt", bufs=2)
                        for ci in range(NCT):
                            nc.sync.dma_start(
                                xt_c[:, ci, :],
                                xT[ci * PT:(ci + 1) * PT,
                                   ch * 512:(ch + 1) * 512])
                        prev = None
                        for u in range(6):
                            zt = _qkv_mm(ch, u, xt_c)
                            if prev is not None:
                                _qkv_consume(ch, *prev)
                            prev = (u, zt)
                        # last unit's stats/rope hide behind attention's start
                        pending = (lambda p=prev: _qkv_consume(ch, *p))
                    if "C" not in ablate:
                        _attn_quad(ch, pending)
                    elif pending is not None:
                        pending()
                    if not timing_only:
                        for c in (2 * ch, 2 * ch + 1):
                            _ag_issue(c)
                    if "P" in ablate:
                        continue
                    for c in range(NCH):
                        if c in done:
                            continue
                        if c <= 2 * ch - 1 or ch == NCH4 - 1:
                            _proj_chunk(c, timing_only)
                            done.append(c)

            if loop_r is None:
                _body(False)
            else:
                with tc.For_i(0, loop_r, 1):
                    _body(True)

    nc.compile()
    return nc


def _tables(q_scale, k_scale):
    inv_freq = THETA ** (-np.arange(0, HS, 2, dtype=np.float64) / HS)
    ang = np.arange(T, dtype=np.float64)[:, None] * inv_freq[None, :]
    cosw = np.concatenate([np.cos(ang), np.cos(ang)], 1)  # (T, 128)
    sinw = np.concatenate([np.sin(ang), np.sin(ang)], 1)
    qs = np.asarray(q_scale, np.float64)
    ks = np.asarray(k_scale, np.float64)
    qs_rot = np.concatenate([qs[H2:], qs[:H2]])
    ks_rot = np.concatenate([ks[H2:], ks[:H2]])
    sgn = np.concatenate([-np.ones(H2), np.ones(H2)])  # -sin on the low half
    s = 1.0 / math.sqrt(HS)
    cosqT = np.ascontiguousarray((cosw * qs[None, :] * s).T).astype(BF)
    sinqT = np.ascontiguousarray((sinw * qs_rot[None, :] * s).T * sgn[:, None]).astype(BF)
    coskT = np.ascontiguousarray((cosw * ks[None, :]).T).astype(BF)
    sinkT = np.ascontiguousarray((sinw * ks_rot[None, :]).T * sgn[:, None]).astype(BF)
    return cosqT, sinqT, coskT, sinkT


def _make_in_maps(x, Wq, Wkv, Wproj, q_scale, k_scale, v1, value_lambda, layer_idx):
    x = np.asarray(x, np.float32)
    Wq = np.asarray(Wq, np.float32)
    Wkv = np.asarray(Wkv, np.float32)
    Wproj = np.asarray(Wproj, np.float32)

    li = int(np.asarray(layer_idx))
    mix = (v1 is not None) and (value_lambda is not None) and li > 0
    lam = float(np.asarray(value_lambda).reshape(())) if mix else 1.0

    cosqT, sinqT, coskT, sinkT = _tables(q_scale, k_scale)
    mneg = (np.tril(np.ones((PT, PT), np.float32), k=-1) * -1e30).astype(BF)

    in_maps = []
    for core in range(NCORES):
        b, r = core // TP, core % TP
        kcols = Wkv[:, r * HS:(r + 1) * HS]
        vcols = Wkv[:, NKV * HS + r * HS: NKV * HS + (r + 1) * HS]
        if mix:
            v1s_np = np.ascontiguousarray(
                ((1.0 - lam) * np.asarray(v1, np.float32)[b, :, r, :]).T
            ).astype(np.float32)
        else:
            v1s_np = np.zeros((HS, T), np.float32)
        in_maps.append({
            "xT": np.ascontiguousarray(x[b].T).astype(BF),
            "wq": Wq[:, r * QW:(r + 1) * QW].astype(BF),
            "wkv": np.ascontiguousarray(np.concatenate([kcols, vcols], 1)).astype(BF),
            "wp": np.ascontiguousarray(Wproj[:, r * QW:(r + 1) * QW]).astype(BF),
            "v1sT": v1s_np,
            "cosqT": cosqT, "sinqT": sinqT, "coskT": coskT, "sinkT": sinkT,
            "mneg": mneg,
        })
    return in_maps


def kernel(x, Wq, Wkv, Wproj, q_scale, k_scale, v1, value_lambda, layer_idx):
    in_maps = _make_in_maps(x, Wq, Wkv, Wproj, q_scale, k_scale, v1,
                            value_lambda, layer_idx)
    if "nc" not in _CACHE:
        _CACHE["nc"] = _build()
    nc = _CACHE["nc"]

    trace = bool(int(os.environ.get("BASS_KERNEL_TRACE", "0")))
    res = run_bass_kernel_spmd(nc, in_maps, core_ids=list(range(NCORES)), trace=trace)
    _CACHE["last"] = res

    y = np.empty((B, T, C), np.float32)
    for core in range(NCORES):
        b, r = core // TP, core % TP
        y[b, :, r * QW:(r + 1) * QW] = np.asarray(res.results[core]["out"])
    return y


# revision 43
# speedup vs baseline: 1.1396x; 1.1396x over previous
"""GQA attention block (RMSNorm-QK + RoPE + causal attention + proj) on 8 TRN2 cores.

Sharding: DP=2 over batch x TP=4 over heads (4 q heads + 1 kv head per core).

v2: chunk-pipelined emission (QKV(ch) -> attention(quad ch) -> AllGather ->
lagged proj) so ACT/DVE streams hide under PE work; v-stationary AV matmuls
producing y^T directly (no output transposes); rowsum via per-tile ones-matmul
(PE) or P_sum accumulation (DVE) per RS_DVE knob; 5-op RoPE with the -sin sign
baked into the host table.
"""

import math
import os
import sys

import numpy as np

for _p in ("/opt/trn_rl_repo", "/root/.axon_site/_ro/trn_rl_repo"):
    if os.path.isdir(_p) and _p not in sys.path:
        sys.path.insert(0, _p)

import ml_dtypes

import concourse.bacc as bacc
import concourse.mybir as mybir
import concourse.tile as tile
from concourse import masks
from concourse.bass_utils import run_bass_kernel_spmd

BF16 = mybir.dt.bfloat16
F32 = mybir.dt.float32
ALU = mybir.AluOpType
AF = mybir.ActivationFunctionType

B, T, C = 2, 2048, 2048
NH, NKV, HS = 16, 4, 128
TP = 4                # tensor-parallel ranks per batch element
QH = NH // TP         # q heads per core
QW = QH * HS          # 512
PT = 128
NT = T // PT          # 16
NCT = C // PT         # 16
NCH4 = 4              # t-chunks of 512
H2 = HS // 2
EPS = 1e-6
THETA = 10000.0
NCORES = 8
BF = ml_dtypes.bfloat16

RS_DVE = int(os.environ.get("BASS_RS_DVE", "2"))  # heads using DVE P_sum rowsum

_CACHE = {}


def _build(loop_r=None, ablate=(), rs_dve=None):
    if rs_dve is None:
        rs_dve = RS_DVE
    nc = bacc.Bacc(None, target_bir_lowering=False, num_devices=NCORES)

    xT = nc.declare_dram_parameter("xT", [C, T], BF16, isOutput=False)
    wq = nc.declare_dram_parameter("wq", [C, QW], BF16, isOutput=False)
    wkv = nc.declare_dram_parameter("wkv", [C, 2 * HS], BF16, isOutput=False)
    wp = nc.declare_dram_parameter("wp", [C, QW], BF16, isOutput=False)
    v1sT = nc.declare_dram_parameter("v1sT", [HS, T], F32, isOutput=False)
    cosqT = nc.declare_dram_parameter("cosqT", [HS, T], BF16, isOutput=False)
    sinqT = nc.declare_dram_parameter("sinqT", [HS, T], BF16, isOutput=False)
    coskT = nc.declare_dram_parameter("coskT", [HS, T], BF16, isOutput=False)
    sinkT = nc.declare_dram_parameter("sinkT", [HS, T], BF16, isOutput=False)
    mneg = nc.declare_dram_parameter("mneg", [PT, PT], BF16, isOutput=False)
    out = nc.declare_dram_parameter("out", [T, QW], BF16, isOutput=True)

    groups = [[0, 1, 2, 3], [4, 5, 6, 7]]

    with tile.TileContext(nc) as tc:
        with (
            tc.tile_pool(name="const", bufs=1) as const,
            tc.tile_pool(name="persist", bufs=1) as persist,
            tc.tile_pool(name="psum", bufs=1, space="PSUM") as psum,
            tc.tile_pool(name="wk", bufs=3) as wk,
            tc.tile_pool(name="dram", bufs=1, space="DRAM") as dram,
        ):
            ident = const.tile([PT, PT], BF16)
            masks.make_identity(nc, ident[:])
            maskt = const.tile([PT, PT], BF16)
            nc.sync.dma_start(maskt[:], mneg[:])
            eps_t = const.tile([PT, 1], F32)
            nc.gpsimd.memset(eps_t[:], EPS)
            ones_t = const.tile([PT, 1], BF16)
            nc.gpsimd.memset(ones_t[:], 1.0)
            ones_f = const.tile([PT, 1], F32)
            nc.gpsimd.memset(ones_f[:], 1.0)
            ones1 = const.tile([1, PT], BF16)
            nc.gpsimd.memset(ones1[:], 1.0)

            # weights + rope tables resident in SBUF; only wkv loads now —
            # the rest is emitted just after ch0's x chunk (DMA-order
            # matters: the single DMA pipe serves requests in issue order)
            wkv_s = persist.tile([PT, NCT, 2 * HS], BF16)
            wq_s = persist.tile([PT, NCT, QW], BF16)
            wp_s = persist.tile([PT, NCT, QW], BF16)
            cq_s = persist.tile([PT, T], BF16)
            sq_s = persist.tile([PT, T], BF16)
            ck_s = persist.tile([PT, T], BF16)
            sk_s = persist.tile([PT, T], BF16)
            for half in range(2):
                nc.sync.dma_start(
                    wkv_s[:, half * 8:(half + 1) * 8, :],
                    wkv[half * 8 * PT:(half + 1) * 8 * PT, :].rearrange(
                        "(c p) w -> p c w", p=PT))

            def _late_loads():
                nc.sync.dma_start(wq_s[:],
                                  wq[:].rearrange("(c p) t -> p c t", p=PT))
                nc.sync.dma_start(ck_s[:], coskT[:])
                nc.sync.dma_start(sk_s[:], sinkT[:])
                nc.sync.dma_start(cq_s[:], cosqT[:])
                nc.sync.dma_start(sq_s[:], sinqT[:])
                nc.sync.dma_start(wp_s[:],
                                  wp[:].rearrange("(c p) t -> p c t", p=PT))

            qT_s = persist.tile([PT, QH, T], BF16)
            kT_s = persist.tile([PT, T], BF16)
            v_s = persist.tile([PT, NT, HS], BF16)
            yT_s = persist.tile([PT, QH, T], BF16)

            TC = 256
            NCH = T // TC  # 8
            ag_ins = [dram.tile([QW, TC], BF16, name=f"ag_in{c}") for c in range(NCH)]
            ag_outs = [dram.tile([C, TC], BF16, name=f"ag_out{c}") for c in range(NCH)]

            def _qkv_mm(ch, u, xt_c):
                """u: 0=k, 1=v, 2..5 = q heads 0..3. Emit the GEMM only."""
                if u == 0:
                    wt, w0 = wkv_s, 0
                elif u == 1:
                    wt, w0 = wkv_s, HS
                else:
                    wt, w0 = wq_s, (u - 2) * HS
                zt = psum.tile([PT, 512], F32, tag="big", bufs=3, name=f"zt{ch}_{u}")
                for ci in range(NCT):
                    nc.tensor.matmul(
                        zt[:], wt[:, ci, w0:w0 + HS], xt_c[:, ci, :],
                        start=(ci == 0), stop=(ci == NCT - 1),
                    )
                return zt

            def _qkv_consume(ch, u, zt, v1tt=None):
                if "R" in ablate:
                    return
                c0 = ch * 512
                if u == 1:
                    # v: residual mix then transpose to keys-on-partitions
                    vmx = wk.tile([PT, 512], BF16, tag="vmx", bufs=2)
                    nc.vector.tensor_tensor(vmx[:], zt[:], v1tt[:], ALU.add)
                    for b in range(4):
                        j = ch * 4 + b
                        tv = psum.tile([PT, PT], BF16, tag="big", bufs=3,
                                       name=f"tv{ch}_{b}")
                        nc.tensor.transpose(tv[:], vmx[:, b * PT:(b + 1) * PT],
                                            ident[:])
                        nc.vector.tensor_copy(v_s[:, j, :], tv[:])
                    return
                # q/k: rmsnorm stats -> scale -> rope
                sqv = wk.tile([PT, 512], BF16, tag="sqv", bufs=2)
                nc.scalar.square(sqv[:], zt[:])
                ms = psum.tile([1, 512], F32, tag="rs", bufs=1, name=f"ms{ch}_{u}")
                nc.tensor.matmul(ms[:], ones_t[:, 0:1], sqv[:], start=True, stop=True)
                rsT = wk.tile([1, 512], F32, tag="rsT", bufs=2)
                nc.scalar.activation(rsT[:], ms[:], AF.Abs_reciprocal_sqrt,
                                     bias=eps_t[0:1, :], scale=1.0 / HS)
                rsS = wk.tile([PT, 512], F32, tag="rsS", bufs=2)
                nc.gpsimd.partition_broadcast(rsS[:], rsT[:])
                zs = wk.tile([PT, 512], BF16, tag="zs", bufs=2)
                nc.vector.tensor_tensor(zs[:], zt[:], rsS[:], ALU.mult)
                ct, st = (ck_s, sk_s) if u == 0 else (cq_s, sq_s)
                kc = wk.tile([PT, 512], BF16, tag="kc", bufs=2)
                nc.vector.tensor_tensor(kc[:], zs[:], ct[:, c0:c0 + 512], ALU.mult)
                zr = wk.tile([PT, 512], BF16, tag="zr", bufs=2)
                nc.vector.tensor_copy(zr[0:H2, :], zs[H2:HS, :])
                nc.vector.tensor_copy(zr[H2:HS, :], zs[0:H2, :])
                tmp = wk.tile([PT, 512], BF16, tag="tmp", bufs=2)
                nc.vector.tensor_tensor(tmp[:], zr[:], st[:, c0:c0 + 512], ALU.mult)
                dst = kT_s[:, c0:c0 + 512] if u == 0 else qT_s[:, u - 2, c0:c0 + 512]
                nc.vector.tensor_tensor(dst, kc[:], tmp[:], ALU.add)

            def _attn_quad(ch, pending=None, fill=None):
                q0 = ch * 512
                J = 4 * ch + 4
                seq = [(h, j) for h in range(QH) for j in range(J)]
                sps = {}
                st8 = {}

                def emit_score(h, j):
                    d = j - 4 * ch
                    w = 512 if d < 0 else (4 - d) * PT
                    qlo = q0 if d < 0 else j * PT
                    sp = psum.tile([PT, 512], F32, tag="big", bufs=3,
                                   name=f"sp{ch}_{h}_{j}")
                    if d >= 0:
                        # causal mask: -1e30 on the strictly-upper block, then
                        # the score matmul accumulates on top
                        nc.tensor.matmul(sp[:, 0:PT], ident[:], maskt[:],
                                         start=True, stop=False,
                                         skip_group_check=True)
                    nc.tensor.matmul(sp[:, 0:w], kT_s[:, j * PT:(j + 1) * PT],
                                     qT_s[:, h, qlo:q0 + 512],
                                     start=(d < 0), stop=True,
                                     skip_group_check=True)
                    sps[(h, j)] = sp

                emit_score(*seq[0])
                if pending is not None:
                    pending()
                if len(seq) > 1:
                    emit_score(*seq[1])
                if fill is not None:
                    fill(12)  # cover the rope-latency gap before scores land
                for idx, (h, j) in enumerate(seq):
                    dve_rs = h >= QH - rs_dve
                    d = j - 4 * ch
                    d0 = max(d, 0)
                    w = 512 if d < 0 else (4 - d) * PT
                    if idx + 2 < len(seq):
                        emit_score(*seq[idx + 2])
                    if fill is not None:
                        fill(1)
                    sp = sps.pop((h, j))
                    if j == 0:
                        yp = psum.tile([PT, 512], F32, tag="yp", bufs=2,
                                       name=f"yp{ch}_{h}")
                        if dve_rs:
                            acc = wk.tile([PT, 512], F32, tag="ps", bufs=2,
                                          name=f"ps{ch}_{h}")
                        else:
                            acc = psum.tile([1, 512], F32, tag="rs", bufs=1,
                                            name=f"rsum{ch}_{h}")
                        st8[h] = (yp, acc)
                    yp, acc = st8[h]
                    pts = wk.tile([PT, 512], BF16, tag="pts", bufs=3)
                    nc.scalar.activation(pts[:, 0:w], sp[:, 0:w], AF.Exp)
                    if d < 0:
                        # full block: single wide AV (+rowsum) matmul
                        nc.tensor.matmul(yp[:], v_s[:, j, :], pts[:],
                                         start=(j == 0), stop=False,
                                         skip_group_check=True)
                        if not dve_rs:
                            nc.tensor.matmul(acc[:], ones_t[:, 0:1], pts[:],
                                             start=(j == 0), stop=False,
                                             skip_group_check=True)
                    else:
                        for r in range(d0, 4):
                            nc.tensor.matmul(
                                yp[:, r * PT:(r + 1) * PT], v_s[:, j, :],
                                pts[:, (r - d0) * PT:(r - d0 + 1) * PT],
                                start=(j == 0 and r == d0),
                                stop=(r == d0),
                                skip_group_check=True,
                            )
                        if not dve_rs:
                            for r in range(d0, 4):
                                nc.tensor.matmul(
                                    acc[:, r * PT:(r + 1) * PT], ones_t[:, 0:1],
                                    pts[:, (r - d0) * PT:(r - d0 + 1) * PT],
                                    start=(j == 0 and r == d0),
                                    stop=(r == d0),
                                    skip_group_check=True,
                                )
                    if dve_rs:
                        if j == 0:
                            nc.vector.tensor_copy(acc[:], pts[:])
                        else:
                            nc.vector.tensor_tensor(
                                acc[:, d0 * PT:], acc[:, d0 * PT:],
                                pts[:, 0:w], ALU.add)
                    if j == J - 1:
                        # finalize head h
                        yp, acc = st8.pop(h)
                        if dve_rs:
                            # stage to bf16: fp32 moving runs at 1/4 rate
                            psb = wk.tile([PT, 512], BF16, tag="psb", bufs=2)
                            nc.vector.tensor_copy(psb[:], acc[:])
                            rsum = psum.tile([1, 512], F32, tag="rs", bufs=1,
                                             name=f"rsum{ch}_{h}")
                            nc.tensor.matmul(rsum[:], ones_t[:, 0:1], psb[:],
                                             start=True, stop=True)
                        else:
                            rsum = acc
                        rinvT = wk.tile([1, 512], F32, tag="rinvT", bufs=2)
                        nc.vector.reciprocal(rinvT[:], rsum[:])
                        rinvS = wk.tile([PT, 512], F32, tag="rsS", bufs=2)
                        nc.gpsimd.partition_broadcast(rinvS[:], rinvT[:])
                        nc.vector.tensor_tensor(yT_s[:, h, q0:q0 + 512], yp[:],
                                                rinvS[:], ALU.mult)

            def _ag_issue(c):
                c0 = c * TC
                nc.sync.dma_start(
                    ag_ins[c][:].rearrange("(h p) t -> p h t", p=PT),
                    yT_s[:, :, c0:c0 + TC],
                )
                nc.gpsimd.collective_compute(
                    "AllGather", ALU.bypass, replica_groups=groups,
                    ins=[ag_ins[c][:]], outs=[ag_outs[c][:]],
                )

            def _proj_chunk_ops(c, timing_only):
                """DMA the gathered chunk now; return deferred per-matmul
                emit closures so proj can interleave as PE filler."""
                ytf = wk.tile([PT, NCT, TC], BF16, tag="ytf", bufs=2,
                              name=f"ytf{c}")
                if timing_only:
                    nc.sync.dma_start(
                        ytf[:],
                        xT[:, c * TC:(c + 1) * TC].rearrange(
                            "(c2 p) t -> p c2 t", p=PT),
                    )
                else:
                    nc.sync.dma_start(
                        ytf[:],
                        ag_outs[c][:].rearrange("(c2 p) t -> p c2 t", p=PT),
                    )
                ops = []
                obox = [None]
                for tt in range(TC // PT):
                    ti = c * (TC // PT) + tt
                    box = [None]

                    def mk(ci, tt=tt, ti=ti, box=box):
                        def go():
                            if ci == 0:
                                box[0] = psum.tile([PT, QW], F32, tag="pp",
                                                   bufs=2, name=f"pp{ti}")
                                if tt == 0:
                                    obox[0] = wk.tile([PT, TC // PT, QW], BF16,
                                                      tag="ot", bufs=2,
                                                      name=f"ot{c}")
                            nc.tensor.matmul(
                                box[0][:], ytf[:, ci, tt * PT:(tt + 1) * PT],
                                wp_s[:, ci, :],
                                start=(ci == 0), stop=(ci == NCT - 1),
                            )
                            if ci == NCT - 1:
                                nc.vector.tensor_copy(obox[0][:, tt, :],
                                                      box[0][:])
                                if tt == TC // PT - 1:
                                    nc.sync.dma_start(
                                        out[c * TC:(c + 1) * TC, :].rearrange(
                                            "(t p) w -> p t w", p=PT),
                                        obox[0][:])
                        return go

                    ops.extend(mk(ci) for ci in range(NCT))
                return ops

            def _body(timing_only):
                filler = []

                def fill(n):
                    for _ in range(min(n, len(filler))):
                        filler.pop(0)()

                def reg(upto):
                    for c in range(upto):
                        if c not in _body.reg:
                            filler.extend(_proj_chunk_ops(c, timing_only))
                            _body.reg.add(c)

                for ch in range(NCH4):
                    pending = None
                    if "A" not in ablate:
                        xt_c = wk.tile([PT, NCT, 512], BF16, tag="xt", bufs=2)
                        for half in range(2):
                            nc.sync.dma_start(
                                xt_c[:, half * 8:(half + 1) * 8, :],
                                xT[half * 8 * PT:(half + 1) * 8 * PT,
                                   ch * 512:(ch + 1) * 512].rearrange(
                                       "(c p) t -> p c t", p=PT))
                        v1tt = wk.tile([PT, 512], F32, tag="v1tt", bufs=2)
                        nc.sync.dma_start(v1tt[:],
                                          v1sT[:, ch * 512:ch * 512 + 512])
                        if ch == 0:
                            _late_loads()
                        prev = None
                        for u in range(6):
                            zt = _qkv_mm(ch, u, xt_c)
                            if prev is not None:
                                _qkv_consume(ch, *prev)
                            prev = (u, zt, v1tt if u == 1 else None)
                        # last unit's stats/rope hide behind attention's start
                        pending = (lambda p=prev: _qkv_consume(ch, *p))
                    if "C" not in ablate:
                        _attn_quad(ch, pending,
                                   fill if "P" not in ablate else None)
                    elif pending is not None:
                        pending()
                    if not timing_only:
                        for c in (2 * ch, 2 * ch + 1):
                            _ag_issue(c)
                    if "P" not in ablate:
                        # prefetch + queue proj work for the next quad
                        reg(2 * ch + 2)
                if "P" not in ablate:
                    reg(NCH)
                    fill(len(filler))  # drain the tail

            _body.reg = set()

            if loop_r is None:
                _body(False)
            elif loop_r == 0:
                # single unrolled timing body (no For_i) for TimelineSim
                _body(True)
            else:
                with tc.For_i(0, loop_r, 1):
                    _body(True)

    nc.compile()
    return nc


def _tables(q_scale, k_scale):
    inv_freq = THETA ** (-np.arange(0, HS, 2, dtype=np.float64) / HS)
    ang = np.arange(T, dtype=np.float64)[:, None] * inv_freq[None, :]
    cosw = np.concatenate([np.cos(ang), np.cos(ang)], 1)  # (T, 128)
    sinw = np.concatenate([np.sin(ang), np.sin(ang)], 1)
    qs = np.asarray(q_scale, np.float64)
    ks = np.asarray(k_scale, np.float64)
    qs_rot = np.concatenate([qs[H2:], qs[:H2]])
    ks_rot = np.concatenate([ks[H2:], ks[:H2]])
    sgn = np.concatenate([-np.ones(H2), np.ones(H2)])  # -sin on the low half
    s = 1.0 / math.sqrt(HS)
    cosqT = np.ascontiguousarray((cosw * qs[None, :] * s).T).astype(BF)
    sinqT = np.ascontiguousarray((sinw * qs_rot[None, :] * s).T * sgn[:, None]).astype(BF)
    coskT = np.ascontiguousarray((cosw * ks[None, :]).T).astype(BF)
    sinkT = np.ascontiguousarray((sinw * ks_rot[None, :]).T * sgn[:, None]).astype(BF)
    return cosqT, sinqT, coskT, sinkT


def _make_in_maps(x, Wq, Wkv, Wproj, q_scale, k_scale, v1, value_lambda, layer_idx):
    x = np.asarray(x, np.float32)
    Wq = np.asarray(Wq, np.float32)
    Wkv = np.asarray(Wkv, np.float32)
    Wproj = np.asarray(Wproj, np.float32)

    li = int(np.asarray(layer_idx))
    mix = (v1 is not None) and (value_lambda is not None) and li > 0
    lam = float(np.asarray(value_lambda).reshape(())) if mix else 1.0

    cosqT, sinqT, coskT, sinkT = _tables(q_scale, k_scale)
    mneg = (np.tril(np.ones((PT, PT), np.float32), k=-1) * -1e30).astype(BF)

    in_maps = []
    for core in range(NCORES):
        b, r = core // TP, core % TP
        kcols = Wkv[:, r * HS:(r + 1) * HS]
        vcols = Wkv[:, NKV * HS + r * HS: NKV * HS + (r + 1) * HS]
        if mix:
            v1s_np = np.ascontiguousarray(
                ((1.0 - lam) * np.asarray(v1, np.float32)[b, :, r, :]).T
            ).astype(np.float32)
        else:
            v1s_np = np.zeros((HS, T), np.float32)
        in_maps.append({
            "xT": np.ascontiguousarray(x[b].T).astype(BF),
            "wq": Wq[:, r * QW:(r + 1) * QW].astype(BF),
            "wkv": np.ascontiguousarray(np.concatenate([kcols, vcols], 1)).astype(BF),
            "wp": np.ascontiguousarray(Wproj[:, r * QW:(r + 1) * QW]).astype(BF),
            "v1sT": v1s_np,
            "cosqT": cosqT, "sinqT": sinqT, "coskT": coskT, "sinkT": sinkT,
            "mneg": mneg,
        })
    return in_maps


def kernel(x, Wq, Wkv, Wproj, q_scale, k_scale, v1, value_lambda, layer_idx):
    in_maps = _make_in_maps(x, Wq, Wkv, Wproj, q_scale, k_scale, v1,
                            value_lambda, layer_idx)
    if "nc" not in _CACHE:
        _CACHE["nc"] = _build()
    nc = _CACHE["nc"]

    trace = bool(int(os.environ.get("BASS_KERNEL_TRACE", "0")))
    res = run_bass_kernel_spmd(nc, in_maps, core_ids=list(range(NCORES)), trace=trace)
    _CACHE["last"] = res

    y = np.empty((B, T, C), np.float32)
    for core in range(NCORES):
        b, r = core // TP, core % TP
        y[b, :, r * QW:(r + 1) * QW] = np.asarray(res.results[core]["out"])
    return y
